# revision 8
# baseline (speedup 1.0000x reference)
"""Trainium2 Bass kernel for masked multi-head attention (nn_Attention_82695300317344).

Reference computation (per (b, h)):
    scores = (Q @ K^T) / sqrt(64)                       # [S, S]
    combined = triu(ones, 1) + padding_mask             # [S, S] (pad broadcast over rows)
    scores = where(combined == 1, -1e9, scores)         # note: overlap (==2) NOT masked
    P = softmax(scores, axis=-1)                        # attention_weights output
    O = P @ V                                           # result output

Sharding: pure data parallel over the 32 (b, h) pairs; core c owns pairs
[4c, 4c+4). All 4 pairs of a core share one batch's padding row.

Per-core device algorithm (S=2048, 4 heads):
  - Masking is fused into the QK^T matmul via augmented contraction rows:
      KA = [K^T ; -240*pad],  KB = [K^T ; -240*(1-pad)],  QA = [Q^T ; ones].
    For t-chunks strictly below the diagonal block the moving operand is KA
    (applies the padding mask); strictly above, KB (applies the future mask
    XOR padding); the 128-wide diagonal tile gets a precomputed additive
    fixup -240*(tri XOR pad) added in PSUM by VectorE.
  - ScalarE computes expS = exp(scores/8) (so -240 becomes the -30 mask)
    with accum_out giving row sums for free; VectorE reciprocal + 2x-mode
    tensor_scalar produce normalized P, written to DRAM as 1 MiB
    contiguous tiles (the 64 MiB/core P output is the memory roofline).
  - expS is kept in bf16; PE transposes 128x128 tiles (bf16 stays bf16 in
    PSUM so the eviction copy runs in the DVE 2x packed mode); the A@V
    matmul runs with V stationary producing O^T (+ its own small PE
    transpose back to [s, d] and per-row normalization).
"""

import numpy as np

import concourse.bass as bass
import concourse.bacc as bacc
import concourse.tile as tile
import concourse.mybir as mybir
from concourse.bass_utils import run_bass_kernel_spmd

dt = mybir.dt
F32 = dt.float32
F32R = dt.float32r
BF16 = dt.bfloat16
I32 = dt.int32
AF = mybir.ActivationFunctionType
ALU = mybir.AluOpType

NEG = -240.0  # additive mask before the exp scale of 1/8 -> -30


def build_nc(S: int = 2048, H: int = 4, D: int = 64):
    """Build + compile the per-core Bass graph.

    Per-core I/O:
      q, k, v : [H, S, D] f32   (H flattened (b,h) pairs of this core)
      pad     : [1, S] i32      (this core's batch padding row)
      p (out) : [H, S, S] f32   (attention weights)
      o (out) : [H, S, D] f32   (attention result)
    """
    assert S % 512 == 0 and D == 64
    NB = S // 128   # number of 128-row s-blocks
    NG = NB // 4    # groups of 4 s-blocks (one 512-wide s-chunk each)
    NT = S // 512   # 512-wide t-chunks
    NH = 2 if NT >= 2 else 1    # PSUM score halves per s-block row
    CPH = NT // NH              # 512-chunks per half
    HWID = S // NH              # columns per half

    from contextlib import ExitStack

    nc = bacc.Bacc(
        "TRN2",
        target_bir_lowering=False,
        debug=False,
        num_devices=8,
    )

    q_d = nc.dram_tensor("q", [H, S, D], F32, kind="ExternalInput").ap()
    k_d = nc.dram_tensor("k", [H, S, D], F32, kind="ExternalInput").ap()
    v_d = nc.dram_tensor("v", [H, S, D], F32, kind="ExternalInput").ap()
    pad_d = nc.dram_tensor("pad", [1, S], I32, kind="ExternalInput").ap()
    p_d = nc.dram_tensor("p", [H, S, S], F32, kind="ExternalOutput").ap()
    o_d = nc.dram_tensor("o", [H, S, D], F32, kind="ExternalOutput").ap()

    ident_d = nc.inline_tensor(np.eye(128, dtype=np.float32), "identconst").ap()
    tri_d = nc.inline_tensor(
        np.triu(np.ones((128, 128), dtype=np.float32), 1), "triconst"
    ).ap()

    with tile.TileContext(nc) as tc, ExitStack() as ctx:
        # ---------------- pools ----------------
        cons = ctx.enter_context(tc.tile_pool(name="cons", bufs=1))
        natp = ctx.enter_context(tc.tile_pool(name="nat", bufs=3))
        kap = ctx.enter_context(tc.tile_pool(name="kap", bufs=2))
        kbp = ctx.enter_context(tc.tile_pool(name="kbp", bufs=2))
        qap = ctx.enter_context(tc.tile_pool(name="qap", bufs=2))
        vbp = ctx.enter_context(tc.tile_pool(name="vbp", bufs=2))
        expsp = ctx.enter_context(tc.tile_pool(name="expsp", bufs=3))
        pnp = ctx.enter_context(tc.tile_pool(name="pnp", bufs=3))
        expstp = ctx.enter_context(tc.tile_pool(name="expstp", bufs=2))
        sumsp = ctx.enter_context(tc.tile_pool(name="sumsp", bufs=10))
        rsump = ctx.enter_context(tc.tile_pool(name="rsump", bufs=10))
        otsbp = ctx.enter_context(tc.tile_pool(name="otsbp", bufs=2))
        obp = ctx.enter_context(tc.tile_pool(name="obp", bufs=3))
        # PSUM pools: sc 3 slots x 2 banks + ot 2 x 1 = 8 banks
        scp = ctx.enter_context(tc.tile_pool(name="scp", bufs=3, space="PSUM"))
        otp_pool = ctx.enter_context(tc.tile_pool(name="otp", bufs=2, space="PSUM"))

        # ---------------- constants / padding prep ----------------
        ident_s = cons.tile([128, 128], F32, tag="ident")
        nc.sync.dma_start(ident_s[:, :], ident_d[:, :])
        tri_s = cons.tile([128, 128], F32, tag="tri")
        nc.sync.dma_start(tri_s[:, :], tri_d[:, :])

        padi = cons.tile([1, S], I32, tag="padi")
        nc.sync.dma_start(padi[:, :], pad_d[:, :])
        padf = cons.tile([1, S], F32, tag="padf")
        nc.vector.tensor_copy(padf[:, :], padi[:, :])

        rowA = cons.tile([1, S], F32R, tag="rowA")
        nc.vector.tensor_scalar(rowA[:, :], padf[:, :], NEG, None, ALU.mult)
        rowB = cons.tile([1, S], F32R, tag="rowB")
        nc.vector.tensor_scalar(rowB[:, :], padf[:, :], -NEG, NEG, ALU.mult, ALU.add)

        ones1 = cons.tile([1, S], F32, tag="ones1")
        nc.gpsimd.memset(ones1[:, :], 1.0)
        onesr = cons.tile([1, S], F32R, tag="onesr")
        nc.vector.tensor_copy(onesr[:, :], ones1[:, :])
        padfr = cons.tile([1, S], F32R, tag="padfr")
        nc.vector.tensor_copy(padfr[:, :], padf[:, :])

        # padb[128, S]: pad row broadcast to 128 partitions (PE outer product)
        padb = pnp.tile([128, S], F32, tag="pn")
        for c in range(NT):
            pb_ps = scp.tile([128, 1024], F32, tag="scp")
            nc.tensor.matmul(
                pb_ps[:, 0:512],
                onesr[0:1, 0:128],
                padfr[0:1, c * 512 : (c + 1) * 512],
                start=True,
                stop=True,
            )
            nc.vector.tensor_copy(padb[:, c * 512 : (c + 1) * 512], pb_ps[:, 0:512])

        # fix[:, i*128:(i+1)*128] = NEG * (tri XOR pad[i*128 + col])
        #   = tri * (NEG - 2*NEG*pad) + NEG*pad
        fix = cons.tile([128, S], F32, tag="fix")
        acoef = pnp.tile([128, S], F32, tag="pn")
        nc.vector.tensor_scalar(
            acoef[:, :], padb[:, :], -2.0 * NEG, NEG, ALU.mult, ALU.add
        )
        bcoef = pnp.tile([128, S], F32, tag="pn")
        nc.vector.tensor_scalar(bcoef[:, :], padb[:, :], NEG, None, ALU.mult)
        for j in range(NB):
            sl = slice(j * 128, (j + 1) * 128)
            nc.vector.tensor_mul(fix[:, sl], tri_s[:, :], acoef[:, sl])
        nc.vector.tensor_add(fix[:, :], fix[:, :], bcoef[:, :])

        # ---------------- per-head main loop ----------------
        for h in range(H):
            # --- K^T prep: KA = [K^T ; rowA], KB = [K^T ; rowB] ---
            knat = natp.tile([128, NB * 64], F32, tag="nat")
            nc.sync.dma_start(
                knat[:, :].rearrange("p (j d) -> p j d", j=NB),
                k_d[h].rearrange("(j p) d -> p j d", p=128),
            )
            ka = kap.tile([65, S], F32R)
            kb = kbp.tile([65, S], F32R)
            for half in range(2):
                nhalf = NB // 2
                ktp = scp.tile([64, nhalf * 128], F32, tag="scp")
                for jj in range(nhalf):
                    j = half * nhalf + jj
                    nc.tensor.transpose(
                        ktp[:, jj * 128 : (jj + 1) * 128],
                        knat[:, j * 64 : (j + 1) * 64],
                        ident_s[:, :],
                    )
                dsl = slice(half * nhalf * 128, (half + 1) * nhalf * 128)
                nc.scalar.copy(ka[0:64, dsl], ktp[:, :])
            nc.sync.dma_start(kb[0:64, :], ka[0:64, :])
            nc.sync.dma_start(ka[64:65, :], rowA[:, :])
            nc.sync.dma_start(kb[64:65, :], rowB[:, :])

            # --- Q^T prep: QA = [Q^T ; ones] ---
            qnat = natp.tile([128, NB * 64], F32, tag="nat")
            nc.sync.dma_start(
                qnat[:, :].rearrange("p (j d) -> p j d", j=NB),
                q_d[h].rearrange("(j p) d -> p j d", p=128),
            )
            qa = qap.tile([65, S], F32R)
            for half in range(2):
                nhalf = NB // 2
                qtp = scp.tile([64, nhalf * 128], F32, tag="scp")
                for jj in range(nhalf):
                    j = half * nhalf + jj
                    nc.tensor.transpose(
                        qtp[:, jj * 128 : (jj + 1) * 128],
                        qnat[:, j * 64 : (j + 1) * 64],
                        ident_s[:, :],
                    )
                dsl = slice(half * nhalf * 128, (half + 1) * nhalf * 128)
                nc.scalar.copy(qa[0:64, dsl], qtp[:, :])
            nc.sync.dma_start(qa[64:65, :], onesr[0:1, :])

            # --- V load + bf16 cast ---
            vnat = natp.tile([128, NB * 64], F32, tag="nat")
            nc.sync.dma_start(
                vnat[:, :].rearrange("p (j d) -> p j d", j=NB),
                v_d[h].rearrange("(j p) d -> p j d", p=128),
            )
            vb = vbp.tile([128, NB * 64], BF16)
            nc.vector.tensor_copy(vb[:, :], vnat[:, :])

            # --- s-block groups ---
            for g in range(NG):
                expst = expstp.tile([128, 4, NB, 128], BF16)
                rsums = []
                for bb in range(4):
                    i = g * 4 + bb
                    s0 = i * 128

                    # scores: NH PSUM halves of [128, HWID]
                    sc = [
                        scp.tile([128, HWID], F32, tag="scp", name=f"sc{hh}_{h}_{i}")
                        for hh in range(NH)
                    ]
                    qa_full = qa[0:65, s0 : s0 + 128]
                    qa_diag = qa[0:64, s0 : s0 + 128]
                    for c in range(NT):
                        half, off = c // CPH, (c % CPH) * 512
                        outc = sc[half][:, off : off + 512]
                        t0 = c * 512
                        if c < g:
                            nc.tensor.matmul(
                                outc,
                                qa_full,
                                ka[0:65, t0 : t0 + 512],
                                start=True,
                                stop=True,
                            )
                        elif c > g:
                            nc.tensor.matmul(
                                outc,
                                qa_full,
                                kb[0:65, t0 : t0 + 512],
                                start=True,
                                stop=True,
                            )
                        else:
                            w1 = s0 - t0  # bb * 128
                            if w1 > 0:
                                nc.tensor.matmul(
                                    outc[:, 0:w1],
                                    qa_full,
                                    ka[0:65, t0 : t0 + w1],
                                    start=True,
                                    stop=True,
                                )
                            nc.tensor.matmul(
                                outc[:, w1 : w1 + 128],
                                qa_diag,
                                ka[0:64, s0 : s0 + 128],
                                start=True,
                                stop=True,
                            )
                            w2 = 512 - w1 - 128
                            if w2 > 0:
                                nc.tensor.matmul(
                                    outc[:, w1 + 128 : 512],
                                    qa_full,
                                    kb[0:65, s0 + 128 : t0 + 512],
                                    start=True,
                                    stop=True,
                                )

                    # diagonal fixup in PSUM
                    dh = g // CPH
                    doff = (g % CPH) * 512 + (s0 - g * 512)
                    nc.vector.tensor_add(
                        sc[dh][:, doff : doff + 128],
                        sc[dh][:, doff : doff + 128],
                        fix[:, i * 128 : (i + 1) * 128],
                    )

                    # exp with accumulated row sums
                    exps = expsp.tile([128, S], BF16)
                    hsums = []
                    for hh in range(NH):
                        hsum = sumsp.tile(
                            [128, 1], F32, tag="hsum", name=f"hsum{hh}_{h}_{i}"
                        )
                        nc.scalar.activation(
                            exps[:, hh * HWID : (hh + 1) * HWID],
                            sc[hh][:, :],
                            AF.Exp,
                            scale=0.125,
                            accum_out=hsum[:, :],
                        )
                        hsums.append(hsum)
                    if NH == 2:
                        sumt = sumsp.tile([128, 1], F32, tag="sumt")
                        nc.vector.tensor_add(
                            sumt[:, :], hsums[0][:, :], hsums[1][:, :]
                        )
                    else:
                        sumt = hsums[0]
                    rsum = rsump.tile([128, 1], F32, tag="rsum")
                    nc.vector.reciprocal(rsum[:, :], sumt[:, :])
                    rsums.append(rsum)

                    # normalize + write P
                    pn = pnp.tile([128, S], F32, tag="pn")
                    nc.vector.tensor_scalar(
                        pn[:, :], exps[:, :], rsum[:, :], None, ALU.mult
                    )
                    nc.sync.dma_start(p_d[h, s0 : s0 + 128, :], pn[:, :])

                    # transpose expS -> expst via the DMA xbar (ACT hwdge ring)
                    nc.scalar.dma_start_transpose(
                        expst[:, bb, :, :], exps[:, :]
                    )

                # A @ V for the 512-wide s-chunk: O^T = sum_j V_j^T @ expst_j
                ot = otp_pool.tile([64, 512], F32, tag="ot")
                for j in range(NB):
                    nc.tensor.matmul(
                        ot[:, :],
                        vb[:, j * 64 : (j + 1) * 64],
                        expst[:, :, j, :],
                        start=(j == 0),
                        stop=(j == NB - 1),
                    )
                otsb = otsbp.tile([64, 512], F32)
                nc.scalar.copy(otsb[:, :], ot[:, :])
                for bb in range(4):
                    i = g * 4 + bb
                    otp = otp_pool.tile([128, 64], F32, tag="ot")
                    nc.tensor.transpose(
                        otp[:, :],
                        otsb[:, bb * 128 : (bb + 1) * 128],
                        ident_s[0:64, 0:64],
                    )
                    ob = obp.tile([128, 64], F32)
                    nc.vector.tensor_scalar(
                        ob[:, :], otp[:, :], rsums[bb][:, :], None, ALU.mult
                    )
                    nc.sync.dma_start(o_d[h, i * 128 : (i + 1) * 128, :], ob[:, :])

    nc.compile()
    return nc


_NC_CACHE: dict = {}


def _get_nc(S: int, H: int, D: int):
    key = (S, H, D)
    if key not in _NC_CACHE:
        _NC_CACHE[key] = build_nc(S, H, D)
    return _NC_CACHE[key]


def _run(q, k, v, padding_mask, trace=False, **kwargs):
    """Shard across 8 cores, run, and reassemble full outputs.

    Returns ((result, attention_weights), BassKernelResults).
    """
    q = np.asarray(q)
    k = np.asarray(k)
    v = np.asarray(v)
    padding_mask = np.asarray(padding_mask)
    B, HH, S, D = q.shape
    n_cores = 8
    hper = (B * HH) // n_cores

    nc = _get_nc(S, hper, D)

    qf = q.reshape(B * HH, S, D)
    kf = k.reshape(B * HH, S, D)
    vf = v.reshape(B * HH, S, D)

    in_maps = []
    for c in range(n_cores):
        b = (c * hper) // HH
        in_maps.append(
            {
                "q": np.ascontiguousarray(qf[c * hper : (c + 1) * hper]),
                "k": np.ascontiguousarray(kf[c * hper : (c + 1) * hper]),
                "v": np.ascontiguousarray(vf[c * hper : (c + 1) * hper]),
                "pad": np.ascontiguousarray(
                    padding_mask[b, 0, 0:1, :].astype(np.int32)
                ),
            }
        )

    res = run_bass_kernel_spmd(
        nc, in_maps, core_ids=list(range(n_cores)), trace=trace, **kwargs
    )
    o_full = np.concatenate([r["o"] for r in res.results], axis=0).reshape(
        B, HH, S, D
    )
    p_full = np.concatenate([r["p"] for r in res.results], axis=0).reshape(
        B, HH, S, S
    )
    return (o_full, p_full), res


def kernel(q, k, v, padding_mask):
    (o_full, p_full), _ = _run(q, k, v, padding_mask)
    return (o_full, p_full)


# revision 9
# speedup vs baseline: 1.6918x; 1.6918x over previous
"""Trainium2 Bass kernel for masked multi-head attention (nn_Attention_82695300317344).

Reference computation (per (b, h)):
    scores = (Q @ K^T) / sqrt(64)                       # [S, S]
    combined = triu(ones, 1) + padding_mask             # [S, S] (pad broadcast over rows)
    scores = where(combined == 1, -1e9, scores)         # note: overlap (==2) NOT masked
    P = softmax(scores, axis=-1)                        # attention_weights output
    O = P @ V                                           # result output

Sharding: pure data parallel over the 32 (b, h) pairs; core c owns pairs
[4c, 4c+4). All 4 pairs of a core share one batch's padding row.

Per-core device algorithm (S=2048, 4 heads):
  - Masking is fused into the QK^T matmul via augmented contraction rows:
      KA = [K^T ; -240*pad],  KB = [K^T ; -240*(1-pad)],  QA = [Q^T ; ones].
    For t-chunks strictly below the diagonal block the moving operand is KA
    (applies the padding mask); strictly above, KB (applies the future mask
    XOR padding); the 128-wide diagonal tile gets a precomputed additive
    fixup -240*(tri XOR pad) added in PSUM by VectorE.
  - ScalarE computes expS = exp(scores/8) (so -240 becomes the -30 mask)
    with accum_out giving row sums for free; VectorE reciprocal + 2x-mode
    tensor_scalar produce normalized P, written to DRAM as 1 MiB
    contiguous tiles (the 64 MiB/core P output is the memory roofline).
  - expS is kept in bf16; PE transposes 128x128 tiles (bf16 stays bf16 in
    PSUM so the eviction copy runs in the DVE 2x packed mode); the A@V
    matmul runs with V stationary producing O^T (+ its own small PE
    transpose back to [s, d] and per-row normalization).
"""

import numpy as np

import concourse.bass as bass
import concourse.bacc as bacc
import concourse.tile as tile
import concourse.mybir as mybir
from concourse.bass_utils import run_bass_kernel_spmd

dt = mybir.dt
F32 = dt.float32
F32R = dt.float32r
BF16 = dt.bfloat16
I32 = dt.int32
AF = mybir.ActivationFunctionType
ALU = mybir.AluOpType

NEG = -240.0  # additive mask before the exp scale of 1/8 -> -30


def build_nc(S: int = 2048, H: int = 4, D: int = 64):
    """Build + compile the per-core Bass graph.

    Per-core I/O:
      q, k, v : [H, S, D] f32   (H flattened (b,h) pairs of this core)
      pad     : [1, S] i32      (this core's batch padding row)
      p (out) : [H, S, S] f32   (attention weights)
      o (out) : [H, S, D] f32   (attention result)
    """
    assert S % 512 == 0 and D == 64
    NB = S // 128   # number of 128-row s-blocks
    NG = NB // 4    # groups of 4 s-blocks (one 512-wide s-chunk each)
    NT = S // 512   # 512-wide t-chunks
    NH = 2 if NT >= 2 else 1    # PSUM score halves per s-block row
    CPH = NT // NH              # 512-chunks per half
    HWID = S // NH              # columns per half

    from contextlib import ExitStack

    nc = bacc.Bacc(
        "TRN2",
        target_bir_lowering=False,
        debug=False,
        num_devices=8,
    )

    q_d = nc.dram_tensor("q", [H, S, D], F32, kind="ExternalInput").ap()
    k_d = nc.dram_tensor("k", [H, S, D], F32, kind="ExternalInput").ap()
    v_d = nc.dram_tensor("v", [H, S, D], F32, kind="ExternalInput").ap()
    pad_d = nc.dram_tensor("pad", [1, S], I32, kind="ExternalInput").ap()
    p_d = nc.dram_tensor("p", [H, S, S], F32, kind="ExternalOutput").ap()
    o_d = nc.dram_tensor("o", [H, S, D], F32, kind="ExternalOutput").ap()

    ident_d = nc.inline_tensor(np.eye(128, dtype=np.float32), "identconst").ap()
    tri_d = nc.inline_tensor(
        np.triu(np.ones((128, 128), dtype=np.float32), 1), "triconst"
    ).ap()

    with tile.TileContext(nc) as tc, ExitStack() as ctx:
        # ---------------- pools ----------------
        cons = ctx.enter_context(tc.tile_pool(name="cons", bufs=1))
        natp = ctx.enter_context(tc.tile_pool(name="nat", bufs=3))
        kap = ctx.enter_context(tc.tile_pool(name="kap", bufs=2))
        kbp = ctx.enter_context(tc.tile_pool(name="kbp", bufs=2))
        qap = ctx.enter_context(tc.tile_pool(name="qap", bufs=2))
        vbp = ctx.enter_context(tc.tile_pool(name="vbp", bufs=2))
        expsp = ctx.enter_context(tc.tile_pool(name="expsp", bufs=3))
        pnp = ctx.enter_context(tc.tile_pool(name="pnp", bufs=3))
        expstp = ctx.enter_context(tc.tile_pool(name="expstp", bufs=2))
        sumsp = ctx.enter_context(tc.tile_pool(name="sumsp", bufs=10))
        rsump = ctx.enter_context(tc.tile_pool(name="rsump", bufs=10))
        otsbp = ctx.enter_context(tc.tile_pool(name="otsbp", bufs=2))
        obp = ctx.enter_context(tc.tile_pool(name="obp", bufs=3))
        # PSUM pools: sc 2 slots x 2 banks + ttb 2 x 1 + ot 2 x 1 = 8 banks
        scp = ctx.enter_context(tc.tile_pool(name="scp", bufs=2, space="PSUM"))
        ttbp = ctx.enter_context(tc.tile_pool(name="ttbp", bufs=2, space="PSUM"))
        otp_pool = ctx.enter_context(tc.tile_pool(name="otp", bufs=2, space="PSUM"))

        # ---------------- constants / padding prep ----------------
        ident_s = cons.tile([128, 128], F32, tag="ident")
        nc.sync.dma_start(ident_s[:, :], ident_d[:, :])
        identb_s = cons.tile([128, 128], BF16, tag="identb")
        nc.vector.tensor_copy(identb_s[:, :], ident_s[:, :])
        tri_s = cons.tile([128, 128], F32, tag="tri")
        nc.sync.dma_start(tri_s[:, :], tri_d[:, :])

        padi = cons.tile([1, S], I32, tag="padi")
        nc.sync.dma_start(padi[:, :], pad_d[:, :])
        padf = cons.tile([1, S], F32, tag="padf")
        nc.vector.tensor_copy(padf[:, :], padi[:, :])

        rowA = cons.tile([1, S], F32R, tag="rowA")
        nc.vector.tensor_scalar(rowA[:, :], padf[:, :], NEG, None, ALU.mult)
        rowB = cons.tile([1, S], F32R, tag="rowB")
        nc.vector.tensor_scalar(rowB[:, :], padf[:, :], -NEG, NEG, ALU.mult, ALU.add)

        ones1 = cons.tile([1, S], F32, tag="ones1")
        nc.gpsimd.memset(ones1[:, :], 1.0)
        onesr = cons.tile([1, S], F32R, tag="onesr")
        nc.vector.tensor_copy(onesr[:, :], ones1[:, :])
        padfr = cons.tile([1, S], F32R, tag="padfr")
        nc.vector.tensor_copy(padfr[:, :], padf[:, :])

        # padb[128, S]: pad row broadcast to 128 partitions (PE outer product)
        padb = pnp.tile([128, S], F32, tag="pn")
        for c in range(NT):
            pb_ps = scp.tile([128, 1024], F32, tag="scp")
            nc.tensor.matmul(
                pb_ps[:, 0:512],
                onesr[0:1, 0:128],
                padfr[0:1, c * 512 : (c + 1) * 512],
                start=True,
                stop=True,
            )
            nc.vector.tensor_copy(padb[:, c * 512 : (c + 1) * 512], pb_ps[:, 0:512])

        # fix[:, i*128:(i+1)*128] = NEG * (tri XOR pad[i*128 + col])
        #   = tri * (NEG - 2*NEG*pad) + NEG*pad
        fix = cons.tile([128, S], F32, tag="fix")
        acoef = pnp.tile([128, S], F32, tag="pn")
        nc.vector.tensor_scalar(
            acoef[:, :], padb[:, :], -2.0 * NEG, NEG, ALU.mult, ALU.add
        )
        bcoef = pnp.tile([128, S], F32, tag="pn")
        nc.vector.tensor_scalar(bcoef[:, :], padb[:, :], NEG, None, ALU.mult)
        for j in range(NB):
            sl = slice(j * 128, (j + 1) * 128)
            nc.vector.tensor_mul(fix[:, sl], tri_s[:, :], acoef[:, sl])
        nc.vector.tensor_add(fix[:, :], fix[:, :], bcoef[:, :])

        # ---------------- per-head main loop ----------------
        for h in range(H):
            # --- K^T prep: KA = [K^T ; rowA], KB = [K^T ; rowB] ---
            knat = natp.tile([128, NB * 64], F32, tag="nat")
            nc.sync.dma_start(
                knat[:, :].rearrange("p (j d) -> p j d", j=NB),
                k_d[h].rearrange("(j p) d -> p j d", p=128),
            )
            ka = kap.tile([65, S], F32R)
            kb = kbp.tile([65, S], F32R)
            for half in range(2):
                nhalf = NB // 2
                ktp = scp.tile([64, nhalf * 128], F32, tag="scp")
                for jj in range(nhalf):
                    j = half * nhalf + jj
                    nc.tensor.transpose(
                        ktp[:, jj * 128 : (jj + 1) * 128],
                        knat[:, j * 64 : (j + 1) * 64],
                        ident_s[:, :],
                    )
                dsl = slice(half * nhalf * 128, (half + 1) * nhalf * 128)
                nc.scalar.copy(ka[0:64, dsl], ktp[:, :])
            nc.sync.dma_start(kb[0:64, :], ka[0:64, :])
            nc.sync.dma_start(ka[64:65, :], rowA[:, :])
            nc.sync.dma_start(kb[64:65, :], rowB[:, :])

            # --- Q^T prep: QA = [Q^T ; ones] ---
            qnat = natp.tile([128, NB * 64], F32, tag="nat")
            nc.sync.dma_start(
                qnat[:, :].rearrange("p (j d) -> p j d", j=NB),
                q_d[h].rearrange("(j p) d -> p j d", p=128),
            )
            qa = qap.tile([65, S], F32R)
            for half in range(2):
                nhalf = NB // 2
                qtp = scp.tile([64, nhalf * 128], F32, tag="scp")
                for jj in range(nhalf):
                    j = half * nhalf + jj
                    nc.tensor.transpose(
                        qtp[:, jj * 128 : (jj + 1) * 128],
                        qnat[:, j * 64 : (j + 1) * 64],
                        ident_s[:, :],
                    )
                dsl = slice(half * nhalf * 128, (half + 1) * nhalf * 128)
                nc.scalar.copy(qa[0:64, dsl], qtp[:, :])
            nc.sync.dma_start(qa[64:65, :], onesr[0:1, :])

            # --- V load + bf16 cast ---
            vnat = natp.tile([128, NB * 64], F32, tag="nat")
            nc.sync.dma_start(
                vnat[:, :].rearrange("p (j d) -> p j d", j=NB),
                v_d[h].rearrange("(j p) d -> p j d", p=128),
            )
            vb = vbp.tile([128, NB * 64], BF16)
            nc.vector.tensor_copy(vb[:, :], vnat[:, :])

            # --- s-block groups ---
            for g in range(NG):
                expst = expstp.tile([128, NB, 512], BF16)
                rsums = []
                for bb in range(4):
                    i = g * 4 + bb
                    s0 = i * 128

                    # scores: NH PSUM halves of [128, HWID]
                    sc = [
                        scp.tile([128, HWID], F32, tag="scp", name=f"sc{hh}_{h}_{i}")
                        for hh in range(NH)
                    ]
                    qa_full = qa[0:65, s0 : s0 + 128]
                    qa_diag = qa[0:64, s0 : s0 + 128]
                    for c in range(NT):
                        half, off = c // CPH, (c % CPH) * 512
                        outc = sc[half][:, off : off + 512]
                        t0 = c * 512
                        if c < g:
                            nc.tensor.matmul(
                                outc,
                                qa_full,
                                ka[0:65, t0 : t0 + 512],
                                start=True,
                                stop=True,
                            )
                        elif c > g:
                            nc.tensor.matmul(
                                outc,
                                qa_full,
                                kb[0:65, t0 : t0 + 512],
                                start=True,
                                stop=True,
                            )
                        else:
                            w1 = s0 - t0  # bb * 128
                            if w1 > 0:
                                nc.tensor.matmul(
                                    outc[:, 0:w1],
                                    qa_full,
                                    ka[0:65, t0 : t0 + w1],
                                    start=True,
                                    stop=True,
                                )
                            nc.tensor.matmul(
                                outc[:, w1 : w1 + 128],
                                qa_diag,
                                ka[0:64, s0 : s0 + 128],
                                start=True,
                                stop=True,
                            )
                            w2 = 512 - w1 - 128
                            if w2 > 0:
                                nc.tensor.matmul(
                                    outc[:, w1 + 128 : 512],
                                    qa_full,
                                    kb[0:65, s0 + 128 : t0 + 512],
                                    start=True,
                                    stop=True,
                                )

                    # diagonal fixup in PSUM
                    dh = g // CPH
                    doff = (g % CPH) * 512 + (s0 - g * 512)
                    nc.vector.tensor_add(
                        sc[dh][:, doff : doff + 128],
                        sc[dh][:, doff : doff + 128],
                        fix[:, i * 128 : (i + 1) * 128],
                    )

                    # exp with accumulated row sums
                    exps = expsp.tile([128, S], BF16)
                    hsums = []
                    for hh in range(NH):
                        hsum = sumsp.tile(
                            [128, 1], F32, tag="hsum", name=f"hsum{hh}_{h}_{i}"
                        )
                        nc.scalar.activation(
                            exps[:, hh * HWID : (hh + 1) * HWID],
                            sc[hh][:, :],
                            AF.Exp,
                            scale=0.125,
                            accum_out=hsum[:, :],
                        )
                        hsums.append(hsum)
                    if NH == 2:
                        sumt = sumsp.tile([128, 1], F32, tag="sumt")
                        nc.vector.tensor_add(
                            sumt[:, :], hsums[0][:, :], hsums[1][:, :]
                        )
                    else:
                        sumt = hsums[0]
                    rsum = rsump.tile([128, 1], F32, tag="rsum")
                    nc.vector.reciprocal(rsum[:, :], sumt[:, :])
                    rsums.append(rsum)

                    # normalize + write P
                    pn = pnp.tile([128, S], F32, tag="pn")
                    nc.vector.tensor_scalar(
                        pn[:, :], exps[:, :], rsum[:, :], None, ALU.mult
                    )
                    nc.sync.dma_start(p_d[h, s0 : s0 + 128, :], pn[:, :])

                    # transpose expS tiles -> expst (bf16 via PSUM)
                    for half in range(2):
                        nhalf = NB // 2
                        ttb = ttbp.tile([128, nhalf * 128], BF16, tag="ttb")
                        for jj in range(nhalf):
                            j = half * nhalf + jj
                            nc.tensor.transpose(
                                ttb[:, jj * 128 : (jj + 1) * 128],
                                exps[:, j * 128 : (j + 1) * 128],
                                identb_s[:, :],
                            )
                        nc.vector.tensor_copy(
                            expst[
                                :,
                                half * nhalf : (half + 1) * nhalf,
                                bb * 128 : (bb + 1) * 128,
                            ],
                            ttb[:, :].rearrange("p (j t) -> p j t", j=nhalf),
                        )

                # A @ V for the 512-wide s-chunk: O^T = sum_j V_j^T @ expst_j
                ot = otp_pool.tile([64, 512], F32, tag="ot")
                for j in range(NB):
                    nc.tensor.matmul(
                        ot[:, :],
                        vb[:, j * 64 : (j + 1) * 64],
                        expst[:, j, :],
                        start=(j == 0),
                        stop=(j == NB - 1),
                    )
                otsb = otsbp.tile([64, 512], F32)
                nc.scalar.copy(otsb[:, :], ot[:, :])
                for bb in range(4):
                    i = g * 4 + bb
                    otp = ttbp.tile([128, 64], F32, tag="ttb")
                    nc.tensor.transpose(
                        otp[:, :],
                        otsb[:, bb * 128 : (bb + 1) * 128],
                        ident_s[0:64, 0:64],
                    )
                    ob = obp.tile([128, 64], F32)
                    nc.vector.tensor_scalar(
                        ob[:, :], otp[:, :], rsums[bb][:, :], None, ALU.mult
                    )
                    nc.sync.dma_start(o_d[h, i * 128 : (i + 1) * 128, :], ob[:, :])

    nc.compile()
    return nc


_NC_CACHE: dict = {}


def _get_nc(S: int, H: int, D: int):
    key = (S, H, D)
    if key not in _NC_CACHE:
        _NC_CACHE[key] = build_nc(S, H, D)
    return _NC_CACHE[key]


def _run(q, k, v, padding_mask, trace=False, **kwargs):
    """Shard across 8 cores, run, and reassemble full outputs.

    Returns ((result, attention_weights), BassKernelResults).
    """
    q = np.asarray(q)
    k = np.asarray(k)
    v = np.asarray(v)
    padding_mask = np.asarray(padding_mask)
    B, HH, S, D = q.shape
    n_cores = 8
    hper = (B * HH) // n_cores

    nc = _get_nc(S, hper, D)

    qf = q.reshape(B * HH, S, D)
    kf = k.reshape(B * HH, S, D)
    vf = v.reshape(B * HH, S, D)

    in_maps = []
    for c in range(n_cores):
        b = (c * hper) // HH
        in_maps.append(
            {
                "q": np.ascontiguousarray(qf[c * hper : (c + 1) * hper]),
                "k": np.ascontiguousarray(kf[c * hper : (c + 1) * hper]),
                "v": np.ascontiguousarray(vf[c * hper : (c + 1) * hper]),
                "pad": np.ascontiguousarray(
                    padding_mask[b, 0, 0:1, :].astype(np.int32)
                ),
            }
        )

    res = run_bass_kernel_spmd(
        nc, in_maps, core_ids=list(range(n_cores)), trace=trace, **kwargs
    )
    o_full = np.concatenate([r["o"] for r in res.results], axis=0).reshape(
        B, HH, S, D
    )
    p_full = np.concatenate([r["p"] for r in res.results], axis=0).reshape(
        B, HH, S, S
    )
    return (o_full, p_full), res


def kernel(q, k, v, padding_mask):
    (o_full, p_full), _ = _run(q, k, v, padding_mask)
    return (o_full, p_full)


# revision 10
# speedup vs baseline: 1.9223x; 1.1363x over previous
"""Trainium2 Bass kernel for masked multi-head attention (nn_Attention_82695300317344).

Reference computation (per (b, h)):
    scores = (Q @ K^T) / sqrt(64)                       # [S, S]
    combined = triu(ones, 1) + padding_mask             # [S, S] (pad broadcast over rows)
    scores = where(combined == 1, -1e9, scores)         # note: overlap (==2) NOT masked
    P = softmax(scores, axis=-1)                        # attention_weights output
    O = P @ V                                           # result output

Sharding: pure data parallel over the 32 (b, h) pairs; core c owns pairs
[4c, 4c+4). All 4 pairs of a core share one batch's padding row.

Per-core device algorithm (S=2048, 4 heads):
  - Masking is fused into the QK^T matmul via augmented contraction rows:
      KA = [K^T ; -240*pad],  KB = [K^T ; -240*(1-pad)],  QA = [Q^T ; ones].
    For t-chunks strictly below the diagonal block the moving operand is KA
    (applies the padding mask); strictly above, KB (applies the future mask
    XOR padding); the 128-wide diagonal tile gets a precomputed additive
    fixup -240*(tri XOR pad) added in PSUM by VectorE.
  - ScalarE computes expS = exp(scores/8) (so -240 becomes the -30 mask)
    with accum_out giving row sums for free; VectorE reciprocal + 2x-mode
    tensor_scalar produce normalized P, written to DRAM as 1 MiB
    contiguous tiles (the 64 MiB/core P output is the memory roofline).
  - expS is kept in bf16; PE transposes 128x128 tiles (bf16 stays bf16 in
    PSUM so the eviction copy runs in the DVE 2x packed mode); the A@V
    matmul runs with V stationary producing O^T (+ its own small PE
    transpose back to [s, d] and per-row normalization).
"""

import numpy as np

import concourse.bass as bass
import concourse.bacc as bacc
import concourse.tile as tile
import concourse.mybir as mybir
from concourse.bass_utils import run_bass_kernel_spmd

dt = mybir.dt
F32 = dt.float32
F32R = dt.float32r
BF16 = dt.bfloat16
I32 = dt.int32
AF = mybir.ActivationFunctionType
ALU = mybir.AluOpType

NEG = -240.0  # additive mask before the exp scale of 1/8 -> -30


def build_nc(S: int = 2048, H: int = 4, D: int = 64):
    """Build + compile the per-core Bass graph.

    Per-core I/O:
      q, k, v : [H, S, D] f32   (H flattened (b,h) pairs of this core)
      pad     : [1, S] i32      (this core's batch padding row)
      p (out) : [H, S, S] f32   (attention weights)
      o (out) : [H, S, D] f32   (attention result)
    """
    assert S % 512 == 0 and D == 64
    NB = S // 128   # number of 128-row s-blocks
    NG = NB // 4    # groups of 4 s-blocks (one 512-wide s-chunk each)
    NT = S // 512   # 512-wide t-chunks
    NH = 2 if NT >= 2 else 1    # PSUM score halves per s-block row
    CPH = NT // NH              # 512-chunks per half
    HWID = S // NH              # columns per half

    from contextlib import ExitStack

    nc = bacc.Bacc(
        "TRN2",
        target_bir_lowering=False,
        debug=False,
        num_devices=8,
    )

    q_d = nc.dram_tensor("q", [H, S, D], F32, kind="ExternalInput").ap()
    k_d = nc.dram_tensor("k", [H, S, D], F32, kind="ExternalInput").ap()
    v_d = nc.dram_tensor("v", [H, S, D], F32, kind="ExternalInput").ap()
    pad_d = nc.dram_tensor("pad", [1, S], I32, kind="ExternalInput").ap()
    p_d = nc.dram_tensor("p", [H, S, S], F32, kind="ExternalOutput").ap()
    o_d = nc.dram_tensor("o", [H, S, D], F32, kind="ExternalOutput").ap()

    ident_d = nc.inline_tensor(np.eye(128, dtype=np.float32), "identconst").ap()
    tri_d = nc.inline_tensor(
        np.triu(np.ones((128, 128), dtype=np.float32), 1), "triconst"
    ).ap()

    with tile.TileContext(nc) as tc, ExitStack() as ctx:
        # ---------------- pools ----------------
        cons = ctx.enter_context(tc.tile_pool(name="cons", bufs=1))
        natp = ctx.enter_context(tc.tile_pool(name="nat", bufs=3))
        kap = ctx.enter_context(tc.tile_pool(name="kap", bufs=2))
        kbp = ctx.enter_context(tc.tile_pool(name="kbp", bufs=2))
        qap = ctx.enter_context(tc.tile_pool(name="qap", bufs=2))
        vbp = ctx.enter_context(tc.tile_pool(name="vbp", bufs=2))
        expsp = ctx.enter_context(tc.tile_pool(name="expsp", bufs=3))
        pnp = ctx.enter_context(tc.tile_pool(name="pnp", bufs=3))
        expstp = ctx.enter_context(tc.tile_pool(name="expstp", bufs=2))
        sumsp = ctx.enter_context(tc.tile_pool(name="sumsp", bufs=10))
        rsump = ctx.enter_context(tc.tile_pool(name="rsump", bufs=10))
        otsbp = ctx.enter_context(tc.tile_pool(name="otsbp", bufs=2))
        obp = ctx.enter_context(tc.tile_pool(name="obp", bufs=3))
        # PSUM pools: sc 2 slots x 2 banks + ttb 2 x 1 + ot 2 x 1 = 8 banks
        scp = ctx.enter_context(tc.tile_pool(name="scp", bufs=2, space="PSUM"))
        ttbp = ctx.enter_context(tc.tile_pool(name="ttbp", bufs=2, space="PSUM"))
        otp_pool = ctx.enter_context(tc.tile_pool(name="otp", bufs=2, space="PSUM"))

        # ---------------- constants / padding prep ----------------
        ident_s = cons.tile([128, 128], F32, tag="ident")
        nc.sync.dma_start(ident_s[:, :], ident_d[:, :])
        identb_s = cons.tile([128, 128], BF16, tag="identb")
        nc.vector.tensor_copy(identb_s[:, :], ident_s[:, :])
        tri_s = cons.tile([128, 128], F32, tag="tri")
        nc.sync.dma_start(tri_s[:, :], tri_d[:, :])

        padi = cons.tile([1, S], I32, tag="padi")
        nc.sync.dma_start(padi[:, :], pad_d[:, :])
        padf = cons.tile([1, S], F32, tag="padf")
        nc.vector.tensor_copy(padf[:, :], padi[:, :])

        rowA = cons.tile([1, S], BF16, tag="rowA")
        nc.vector.tensor_scalar(rowA[:, :], padf[:, :], NEG, None, ALU.mult)
        rowB = cons.tile([1, S], BF16, tag="rowB")
        nc.vector.tensor_scalar(rowB[:, :], padf[:, :], -NEG, NEG, ALU.mult, ALU.add)

        onesb = cons.tile([1, S], BF16, tag="onesb")
        nc.gpsimd.memset(onesb[:, :], 1.0)
        padfb = cons.tile([1, S], BF16, tag="padfb")
        nc.vector.tensor_copy(padfb[:, :], padf[:, :])

        # padb[128, S]: pad row broadcast to 128 partitions (PE outer product)
        padb = pnp.tile([128, S], F32, tag="pn")
        for c in range(NT):
            pb_ps = scp.tile([128, 1024], F32, tag="scp")
            nc.tensor.matmul(
                pb_ps[:, 0:512],
                onesb[0:1, 0:128],
                padfb[0:1, c * 512 : (c + 1) * 512],
                start=True,
                stop=True,
            )
            nc.vector.tensor_copy(padb[:, c * 512 : (c + 1) * 512], pb_ps[:, 0:512])

        # fix[:, i*128:(i+1)*128] = NEG * (tri XOR pad[i*128 + col])
        #   = tri * (NEG - 2*NEG*pad) + NEG*pad
        fix = cons.tile([128, S], F32, tag="fix")
        acoef = pnp.tile([128, S], F32, tag="pn")
        nc.vector.tensor_scalar(
            acoef[:, :], padb[:, :], -2.0 * NEG, NEG, ALU.mult, ALU.add
        )
        bcoef = pnp.tile([128, S], F32, tag="pn")
        nc.vector.tensor_scalar(bcoef[:, :], padb[:, :], NEG, None, ALU.mult)
        for j in range(NB):
            sl = slice(j * 128, (j + 1) * 128)
            nc.vector.tensor_mul(fix[:, sl], tri_s[:, :], acoef[:, sl])
        nc.vector.tensor_add(fix[:, :], fix[:, :], bcoef[:, :])

        # ---------------- per-head main loop ----------------
        for h in range(H):
            # --- K^T prep: KA = [K^T ; rowA], KB = [K^T ; rowB] ---
            knat = natp.tile([128, NB * 64], F32, tag="nat")
            nc.sync.dma_start(
                knat[:, :].rearrange("p (j d) -> p j d", j=NB),
                k_d[h].rearrange("(j p) d -> p j d", p=128),
            )
            knb = natp.tile([128, NB * 64], BF16, tag="natb")
            nc.vector.tensor_copy(knb[:, :], knat[:, :])
            ka = kap.tile([65, S], BF16)
            kb = kbp.tile([65, S], BF16)
            for half in range(2):
                nhalf = NB // 2
                ktp = ttbp.tile([64, nhalf * 128], BF16, tag="ttb")
                for jj in range(nhalf):
                    j = half * nhalf + jj
                    nc.tensor.transpose(
                        ktp[:, jj * 128 : (jj + 1) * 128],
                        knb[:, j * 64 : (j + 1) * 64],
                        identb_s[:, :],
                    )
                dsl = slice(half * nhalf * 128, (half + 1) * nhalf * 128)
                nc.scalar.copy(ka[0:64, dsl], ktp[:, :])
            nc.sync.dma_start(kb[0:64, :], ka[0:64, :])
            nc.sync.dma_start(ka[64:65, :], rowA[:, :])
            nc.sync.dma_start(kb[64:65, :], rowB[:, :])

            # --- Q^T prep: QA = [Q^T ; ones] ---
            qnat = natp.tile([128, NB * 64], F32, tag="nat")
            nc.sync.dma_start(
                qnat[:, :].rearrange("p (j d) -> p j d", j=NB),
                q_d[h].rearrange("(j p) d -> p j d", p=128),
            )
            qnb = natp.tile([128, NB * 64], BF16, tag="natb")
            nc.vector.tensor_copy(qnb[:, :], qnat[:, :])
            qa = qap.tile([65, S], BF16)
            for half in range(2):
                nhalf = NB // 2
                qtp = ttbp.tile([64, nhalf * 128], BF16, tag="ttb")
                for jj in range(nhalf):
                    j = half * nhalf + jj
                    nc.tensor.transpose(
                        qtp[:, jj * 128 : (jj + 1) * 128],
                        qnb[:, j * 64 : (j + 1) * 64],
                        identb_s[:, :],
                    )
                dsl = slice(half * nhalf * 128, (half + 1) * nhalf * 128)
                nc.scalar.copy(qa[0:64, dsl], qtp[:, :])
            nc.sync.dma_start(qa[64:65, :], onesb[0:1, :])

            # --- V load + bf16 cast ---
            vnat = natp.tile([128, NB * 64], F32, tag="nat")
            nc.sync.dma_start(
                vnat[:, :].rearrange("p (j d) -> p j d", j=NB),
                v_d[h].rearrange("(j p) d -> p j d", p=128),
            )
            vb = vbp.tile([128, NB * 64], BF16)
            nc.vector.tensor_copy(vb[:, :], vnat[:, :])

            # --- s-block groups ---
            for g in range(NG):
                expst = expstp.tile([128, NB, 512], BF16)
                rsums = []
                for bb in range(4):
                    i = g * 4 + bb
                    s0 = i * 128

                    # scores: NH PSUM halves of [128, HWID]
                    sc = [
                        scp.tile([128, HWID], F32, tag="scp", name=f"sc{hh}_{h}_{i}")
                        for hh in range(NH)
                    ]
                    qa_full = qa[0:65, s0 : s0 + 128]
                    qa_diag = qa[0:64, s0 : s0 + 128]
                    for c in range(NT):
                        half, off = c // CPH, (c % CPH) * 512
                        outc = sc[half][:, off : off + 512]
                        t0 = c * 512
                        if c < g:
                            nc.tensor.matmul(
                                outc,
                                qa_full,
                                ka[0:65, t0 : t0 + 512],
                                start=True,
                                stop=True,
                            )
                        elif c > g:
                            nc.tensor.matmul(
                                outc,
                                qa_full,
                                kb[0:65, t0 : t0 + 512],
                                start=True,
                                stop=True,
                            )
                        else:
                            w1 = s0 - t0  # bb * 128
                            if w1 > 0:
                                nc.tensor.matmul(
                                    outc[:, 0:w1],
                                    qa_full,
                                    ka[0:65, t0 : t0 + w1],
                                    start=True,
                                    stop=True,
                                )
                            nc.tensor.matmul(
                                outc[:, w1 : w1 + 128],
                                qa_diag,
                                ka[0:64, s0 : s0 + 128],
                                start=True,
                                stop=True,
                            )
                            w2 = 512 - w1 - 128
                            if w2 > 0:
                                nc.tensor.matmul(
                                    outc[:, w1 + 128 : 512],
                                    qa_full,
                                    kb[0:65, s0 + 128 : t0 + 512],
                                    start=True,
                                    stop=True,
                                )

                    # diagonal fixup in PSUM
                    dh = g // CPH
                    doff = (g % CPH) * 512 + (s0 - g * 512)
                    nc.vector.tensor_add(
                        sc[dh][:, doff : doff + 128],
                        sc[dh][:, doff : doff + 128],
                        fix[:, i * 128 : (i + 1) * 128],
                    )

                    # exp with accumulated row sums
                    exps = expsp.tile([128, S], BF16)
                    hsums = []
                    for hh in range(NH):
                        hsum = sumsp.tile(
                            [128, 1], F32, tag="hsum", name=f"hsum{hh}_{h}_{i}"
                        )
                        nc.scalar.activation(
                            exps[:, hh * HWID : (hh + 1) * HWID],
                            sc[hh][:, :],
                            AF.Exp,
                            scale=0.125,
                            accum_out=hsum[:, :],
                        )
                        hsums.append(hsum)
                    if NH == 2:
                        sumt = sumsp.tile([128, 1], F32, tag="sumt")
                        nc.vector.tensor_add(
                            sumt[:, :], hsums[0][:, :], hsums[1][:, :]
                        )
                    else:
                        sumt = hsums[0]
                    rsum = rsump.tile([128, 1], F32, tag="rsum")
                    nc.vector.reciprocal(rsum[:, :], sumt[:, :])
                    rsums.append(rsum)

                    # normalize + write P
                    pn = pnp.tile([128, S], F32, tag="pn")
                    nc.vector.tensor_scalar(
                        pn[:, :], exps[:, :], rsum[:, :], None, ALU.mult
                    )
                    nc.sync.dma_start(p_d[h, s0 : s0 + 128, :], pn[:, :])

                    # transpose expS tiles -> expst (bf16 via PSUM)
                    for half in range(2):
                        nhalf = NB // 2
                        ttb = ttbp.tile([128, nhalf * 128], BF16, tag="ttb")
                        for jj in range(nhalf):
                            j = half * nhalf + jj
                            nc.tensor.transpose(
                                ttb[:, jj * 128 : (jj + 1) * 128],
                                exps[:, j * 128 : (j + 1) * 128],
                                identb_s[:, :],
                            )
                        nc.vector.tensor_copy(
                            expst[
                                :,
                                half * nhalf : (half + 1) * nhalf,
                                bb * 128 : (bb + 1) * 128,
                            ],
                            ttb[:, :].rearrange("p (j t) -> p j t", j=nhalf),
                        )

                # A @ V for the 512-wide s-chunk: O^T = sum_j V_j^T @ expst_j
                ot = otp_pool.tile([64, 512], F32, tag="ot")
                for j in range(NB):
                    nc.tensor.matmul(
                        ot[:, :],
                        vb[:, j * 64 : (j + 1) * 64],
                        expst[:, j, :],
                        start=(j == 0),
                        stop=(j == NB - 1),
                    )
                otsb = otsbp.tile([64, 512], BF16)
                nc.scalar.copy(otsb[:, :], ot[:, :])
                for bb in range(4):
                    i = g * 4 + bb
                    otp = ttbp.tile([128, 64], BF16, tag="ttb")
                    nc.tensor.transpose(
                        otp[:, :],
                        otsb[:, bb * 128 : (bb + 1) * 128],
                        identb_s[0:64, 0:64],
                    )
                    ob = obp.tile([128, 64], F32)
                    nc.vector.tensor_scalar(
                        ob[:, :], otp[:, :], rsums[bb][:, :], None, ALU.mult
                    )
                    nc.sync.dma_start(o_d[h, i * 128 : (i + 1) * 128, :], ob[:, :])

    nc.compile()
    return nc


_NC_CACHE: dict = {}


def _get_nc(S: int, H: int, D: int):
    key = (S, H, D)
    if key not in _NC_CACHE:
        _NC_CACHE[key] = build_nc(S, H, D)
    return _NC_CACHE[key]


def _run(q, k, v, padding_mask, trace=False, **kwargs):
    """Shard across 8 cores, run, and reassemble full outputs.

    Returns ((result, attention_weights), BassKernelResults).
    """
    q = np.asarray(q)
    k = np.asarray(k)
    v = np.asarray(v)
    padding_mask = np.asarray(padding_mask)
    B, HH, S, D = q.shape
    n_cores = 8
    hper = (B * HH) // n_cores

    nc = _get_nc(S, hper, D)

    qf = q.reshape(B * HH, S, D)
    kf = k.reshape(B * HH, S, D)
    vf = v.reshape(B * HH, S, D)

    in_maps = []
    for c in range(n_cores):
        b = (c * hper) // HH
        in_maps.append(
            {
                "q": np.ascontiguousarray(qf[c * hper : (c + 1) * hper]),
                "k": np.ascontiguousarray(kf[c * hper : (c + 1) * hper]),
                "v": np.ascontiguousarray(vf[c * hper : (c + 1) * hper]),
                "pad": np.ascontiguousarray(
                    padding_mask[b, 0, 0:1, :].astype(np.int32)
                ),
            }
        )

    res = run_bass_kernel_spmd(
        nc, in_maps, core_ids=list(range(n_cores)), trace=trace, **kwargs
    )
    o_full = np.concatenate([r["o"] for r in res.results], axis=0).reshape(
        B, HH, S, D
    )
    p_full = np.concatenate([r["p"] for r in res.results], axis=0).reshape(
        B, HH, S, S
    )
    return (o_full, p_full), res


def kernel(q, k, v, padding_mask):
    (o_full, p_full), _ = _run(q, k, v, padding_mask)
    return (o_full, p_full)


# revision 12
# speedup vs baseline: 2.1082x; 1.0967x over previous
"""Trainium2 Bass kernel for masked multi-head attention (nn_Attention_82695300317344).

Reference computation (per (b, h)):
    scores = (Q @ K^T) / sqrt(64)                       # [S, S]
    combined = triu(ones, 1) + padding_mask             # [S, S] (pad broadcast over rows)
    scores = where(combined == 1, -1e9, scores)         # note: overlap (==2) NOT masked
    P = softmax(scores, axis=-1)                        # attention_weights output
    O = P @ V                                           # result output

Sharding: pure data parallel over the 32 (b, h) pairs; core c owns pairs
[4c, 4c+4). All 4 pairs of a core share one batch's padding row.

Per-core device algorithm (S=2048, 4 heads):
  - Masking is fused into the QK^T matmul via augmented contraction rows:
      KA = [K^T ; -240*pad],  KB = [K^T ; -240*(1-pad)],  QA = [Q^T ; ones].
    For t-chunks strictly below the diagonal block the moving operand is KA
    (applies the padding mask); strictly above, KB (applies the future mask
    XOR padding); the 128-wide diagonal tile gets a precomputed additive
    fixup -240*(tri XOR pad) added in PSUM by VectorE.
  - ScalarE computes expS = exp(scores/8) (so -240 becomes the -30 mask)
    with accum_out giving row sums for free; VectorE reciprocal + 2x-mode
    tensor_scalar produce normalized P, written to DRAM as 1 MiB
    contiguous tiles (the 64 MiB/core P output is the memory roofline).
  - expS is kept in bf16; PE transposes 128x128 tiles (bf16 stays bf16 in
    PSUM so the eviction copy runs in the DVE 2x packed mode); the A@V
    matmul runs with V stationary producing O^T (+ its own small PE
    transpose back to [s, d] and per-row normalization).
"""

import numpy as np

import concourse.bass as bass
import concourse.bacc as bacc
import concourse.tile as tile
import concourse.mybir as mybir
from concourse.bass_utils import run_bass_kernel_spmd

dt = mybir.dt
F32 = dt.float32
F32R = dt.float32r
BF16 = dt.bfloat16
I32 = dt.int32
AF = mybir.ActivationFunctionType
ALU = mybir.AluOpType

NEG = -240.0  # additive mask before the exp scale of 1/8 -> -30


def build_nc(S: int = 2048, H: int = 4, D: int = 64):
    """Build + compile the per-core Bass graph.

    Per-core I/O:
      q, k, v : [H, S, D] f32   (H flattened (b,h) pairs of this core)
      pad     : [1, S] i32      (this core's batch padding row)
      p (out) : [H, S, S] f32   (attention weights)
      o (out) : [H, S, D] f32   (attention result)
    """
    assert S % 512 == 0 and D == 64
    NB = S // 128   # number of 128-row s-blocks
    NG = NB // 4    # groups of 4 s-blocks (one 512-wide s-chunk each)
    NT = S // 512   # 512-wide t-chunks
    NH = 2 if NT >= 2 else 1    # PSUM score halves per s-block row
    CPH = NT // NH              # 512-chunks per half
    HWID = S // NH              # columns per half

    from contextlib import ExitStack

    nc = bacc.Bacc(
        "TRN2",
        target_bir_lowering=False,
        debug=False,
        num_devices=8,
    )

    q_d = nc.dram_tensor("q", [H, S, D], F32, kind="ExternalInput").ap()
    k_d = nc.dram_tensor("k", [H, S, D], F32, kind="ExternalInput").ap()
    v_d = nc.dram_tensor("v", [H, S, D], F32, kind="ExternalInput").ap()
    pad_d = nc.dram_tensor("pad", [1, S], I32, kind="ExternalInput").ap()
    p_d = nc.dram_tensor("p", [H, S, S], F32, kind="ExternalOutput").ap()
    o_d = nc.dram_tensor("o", [H, S, D], F32, kind="ExternalOutput").ap()

    ident_d = nc.inline_tensor(np.eye(128, dtype=np.float32), "identconst").ap()
    tri_d = nc.inline_tensor(
        np.triu(np.ones((128, 128), dtype=np.float32), 1), "triconst"
    ).ap()

    with tile.TileContext(nc) as tc, ExitStack() as ctx:
        # ---------------- pools ----------------
        cons = ctx.enter_context(tc.tile_pool(name="cons", bufs=1))
        natp = ctx.enter_context(tc.tile_pool(name="nat", bufs=3))
        kap = ctx.enter_context(tc.tile_pool(name="kap", bufs=2))
        kbp = ctx.enter_context(tc.tile_pool(name="kbp", bufs=2))
        qap = ctx.enter_context(tc.tile_pool(name="qap", bufs=2))
        vbp = ctx.enter_context(tc.tile_pool(name="vbp", bufs=2))
        expsp = ctx.enter_context(tc.tile_pool(name="expsp", bufs=3))
        pnp = ctx.enter_context(tc.tile_pool(name="pnp", bufs=3))
        expstp = ctx.enter_context(tc.tile_pool(name="expstp", bufs=2))
        sumsp = ctx.enter_context(tc.tile_pool(name="sumsp", bufs=10))
        rsump = ctx.enter_context(tc.tile_pool(name="rsump", bufs=10))
        otsbp = ctx.enter_context(tc.tile_pool(name="otsbp", bufs=2))
        obp = ctx.enter_context(tc.tile_pool(name="obp", bufs=3))
        # PSUM pools: sc 2 slots x 2 banks + ttb 2 x 1 + ot 2 x 1 = 8 banks
        scp = ctx.enter_context(tc.tile_pool(name="scp", bufs=2, space="PSUM"))
        ttbp = ctx.enter_context(tc.tile_pool(name="ttbp", bufs=2, space="PSUM"))
        otp_pool = ctx.enter_context(tc.tile_pool(name="otp", bufs=2, space="PSUM"))

        # ---------------- constants / padding prep ----------------
        ident_s = cons.tile([128, 128], F32, tag="ident")
        nc.sync.dma_start(ident_s[:, :], ident_d[:, :])
        identb_s = cons.tile([128, 128], BF16, tag="identb")
        nc.vector.tensor_copy(identb_s[:, :], ident_s[:, :])
        tri_s = cons.tile([128, 128], F32, tag="tri")
        nc.sync.dma_start(tri_s[:, :], tri_d[:, :])

        padi = cons.tile([1, S], I32, tag="padi")
        nc.sync.dma_start(padi[:, :], pad_d[:, :])
        padf = cons.tile([1, S], F32, tag="padf")
        nc.vector.tensor_copy(padf[:, :], padi[:, :])

        rowA = cons.tile([1, S], BF16, tag="rowA")
        nc.vector.tensor_scalar(rowA[:, :], padf[:, :], NEG, None, ALU.mult)
        rowB = cons.tile([1, S], BF16, tag="rowB")
        nc.vector.tensor_scalar(rowB[:, :], padf[:, :], -NEG, NEG, ALU.mult, ALU.add)

        onesb = cons.tile([1, S], BF16, tag="onesb")
        nc.gpsimd.memset(onesb[:, :], 1.0)
        padfb = cons.tile([1, S], BF16, tag="padfb")
        nc.vector.tensor_copy(padfb[:, :], padf[:, :])

        # padb[128, S]: pad row broadcast to 128 partitions (PE outer product)
        padb = pnp.tile([128, S], F32, tag="pn")
        for c in range(NT):
            pb_ps = scp.tile([128, 1024], F32, tag="scp")
            nc.tensor.matmul(
                pb_ps[:, 0:512],
                onesb[0:1, 0:128],
                padfb[0:1, c * 512 : (c + 1) * 512],
                start=True,
                stop=True,
            )
            nc.vector.tensor_copy(padb[:, c * 512 : (c + 1) * 512], pb_ps[:, 0:512])

        # Diagonal-chunk corrections. The diag 512-chunk g is computed with
        # KB (rowB mask) for blocks bb in {0,1} and KA (rowA) for bb in {2,3};
        # the correction added in PSUM is u[t]=240*(1-2*pad[t]) over t<=s
        # (KB case) or -u[t] over t>s (KA case), restricted to the column
        # range where the applied row is wrong.
        ub = cons.tile([128, S], F32, tag="ub")
        nc.vector.tensor_scalar(ub[:, :], padb[:, :], -480.0, 240.0, ALU.mult, ALU.add)
        nub = cons.tile([128, S], F32, tag="nub")
        nc.vector.tensor_scalar(nub[:, :], padb[:, :], 480.0, -240.0, ALU.mult, ALU.add)
        ltri = cons.tile([128, 128], F32, tag="ltri")
        nc.vector.tensor_scalar(ltri[:, :], tri_s[:, :], -1.0, 1.0, ALU.mult, ALU.add)
        # fixv layout per group g (offset g*768):
        #   bb=0: [0,128)   = ltri * u[512g:+128]          (targets chunk cols [0,128))
        #   bb=1: [128,384) = [u[512g:+128] | ltri*u[512g+128:+128]]   (cols [0,256))
        #   bb=2: [384,640) = [tri*(-u[512g+256:+128]) | -u[512g+384:+128]] (cols [256,512))
        #   bb=3: [640,768) = tri*(-u[512g+384:+128])      (cols [384,512))
        fixv = cons.tile([128, NG * 768], F32, tag="fixv")
        for g in range(NG):
            t0 = g * 512
            fo = g * 768
            nc.vector.tensor_mul(
                fixv[:, fo : fo + 128], ltri[:, :], ub[:, t0 : t0 + 128]
            )
            nc.vector.tensor_copy(
                fixv[:, fo + 128 : fo + 256], ub[:, t0 : t0 + 128]
            )
            nc.vector.tensor_mul(
                fixv[:, fo + 256 : fo + 384], ltri[:, :], ub[:, t0 + 128 : t0 + 256]
            )
            nc.vector.tensor_mul(
                fixv[:, fo + 384 : fo + 512], tri_s[:, :], nub[:, t0 + 256 : t0 + 384]
            )
            nc.vector.tensor_copy(
                fixv[:, fo + 512 : fo + 640], nub[:, t0 + 384 : t0 + 512]
            )
            nc.vector.tensor_mul(
                fixv[:, fo + 640 : fo + 768], tri_s[:, :], nub[:, t0 + 384 : t0 + 512]
            )

        # ---------------- per-head main loop ----------------
        for h in range(H):
            # --- K^T prep: KA = [K^T ; rowA], KB = [K^T ; rowB] ---
            knat = natp.tile([128, NB * 64], F32, tag="nat")
            nc.sync.dma_start(
                knat[:, :].rearrange("p (j d) -> p j d", j=NB),
                k_d[h].rearrange("(j p) d -> p j d", p=128),
            )
            knb = natp.tile([128, NB * 64], BF16, tag="natb")
            nc.vector.tensor_copy(knb[:, :], knat[:, :])
            ka = kap.tile([65, S], BF16)
            kb = kbp.tile([65, S], BF16)
            for half in range(2):
                nhalf = NB // 2
                ktp = ttbp.tile([64, nhalf * 128], BF16, tag="ttb")
                for jj in range(nhalf):
                    j = half * nhalf + jj
                    nc.tensor.transpose(
                        ktp[:, jj * 128 : (jj + 1) * 128],
                        knb[:, j * 64 : (j + 1) * 64],
                        identb_s[:, :],
                    )
                dsl = slice(half * nhalf * 128, (half + 1) * nhalf * 128)
                nc.scalar.copy(ka[0:64, dsl], ktp[:, :])
            nc.sync.dma_start(kb[0:64, :], ka[0:64, :])
            nc.sync.dma_start(ka[64:65, :], rowA[:, :])
            nc.sync.dma_start(kb[64:65, :], rowB[:, :])

            # --- Q^T prep: QA = [Q^T ; ones] ---
            qnat = natp.tile([128, NB * 64], F32, tag="nat")
            nc.sync.dma_start(
                qnat[:, :].rearrange("p (j d) -> p j d", j=NB),
                q_d[h].rearrange("(j p) d -> p j d", p=128),
            )
            qnb = natp.tile([128, NB * 64], BF16, tag="natb")
            nc.vector.tensor_copy(qnb[:, :], qnat[:, :])
            qa = qap.tile([65, S], BF16)
            for half in range(2):
                nhalf = NB // 2
                qtp = ttbp.tile([64, nhalf * 128], BF16, tag="ttb")
                for jj in range(nhalf):
                    j = half * nhalf + jj
                    nc.tensor.transpose(
                        qtp[:, jj * 128 : (jj + 1) * 128],
                        qnb[:, j * 64 : (j + 1) * 64],
                        identb_s[:, :],
                    )
                dsl = slice(half * nhalf * 128, (half + 1) * nhalf * 128)
                nc.scalar.copy(qa[0:64, dsl], qtp[:, :])
            nc.sync.dma_start(qa[64:65, :], onesb[0:1, :])

            # --- V load + bf16 cast ---
            vnat = natp.tile([128, NB * 64], F32, tag="nat")
            nc.sync.dma_start(
                vnat[:, :].rearrange("p (j d) -> p j d", j=NB),
                v_d[h].rearrange("(j p) d -> p j d", p=128),
            )
            vb = vbp.tile([128, NB * 64], BF16)
            nc.vector.tensor_copy(vb[:, :], vnat[:, :])

            # --- s-block main loop (1-block software skew on PE) ---
            # per-block state carried across the skew
            state = {}

            def emit_scores(i):
                g, bb = i // 4, i % 4
                s0 = i * 128
                sc = [
                    scp.tile([128, HWID], F32, tag="scp", name=f"sc{hh}_{h}_{i}")
                    for hh in range(NH)
                ]
                qa_full = qa[0:65, s0 : s0 + 128]
                for c in range(NT):
                    half, off = c // CPH, (c % CPH) * 512
                    outc = sc[half][:, off : off + 512]
                    t0 = c * 512
                    use_ka = (c < g) or (c == g and bb >= 2)
                    src_k = ka if use_ka else kb
                    nc.tensor.matmul(
                        outc,
                        qa_full,
                        src_k[0:65, t0 : t0 + 512],
                        start=True,
                        stop=True,
                    )
                # diagonal-chunk correction in PSUM
                FIXO = {0: (0, 128, 0), 1: (128, 256, 0), 2: (384, 256, 256), 3: (640, 128, 384)}
                fo, w, cstart = FIXO[bb]
                fo += g * 768
                dh = g // CPH
                doff = (g % CPH) * 512 + cstart
                nc.vector.tensor_add(
                    sc[dh][:, doff : doff + w],
                    sc[dh][:, doff : doff + w],
                    fixv[:, fo : fo + w],
                )
                # exp with accumulated row sums
                exps = expsp.tile([128, S], BF16, tag="exps", name=f"exps_{h}_{i}")
                hsums = []
                for hh in range(NH):
                    hsum = sumsp.tile(
                        [128, 1], F32, tag="hsum", name=f"hsum{hh}_{h}_{i}"
                    )
                    nc.scalar.activation(
                        exps[:, hh * HWID : (hh + 1) * HWID],
                        sc[hh][:, :],
                        AF.Exp,
                        scale=0.125,
                        accum_out=hsum[:, :],
                    )
                    hsums.append(hsum)
                if NH == 2:
                    sumt = sumsp.tile([128, 1], F32, tag="sumt", name=f"sumt_{h}_{i}")
                    nc.vector.tensor_add(sumt[:, :], hsums[0][:, :], hsums[1][:, :])
                else:
                    sumt = hsums[0]
                rsum = rsump.tile([128, 1], F32, tag="rsum", name=f"rsum_{h}_{i}")
                nc.vector.reciprocal(rsum[:, :], sumt[:, :])
                state[i] = (exps, rsum)

            def emit_post(i, expst):
                g, bb = i // 4, i % 4
                s0 = i * 128
                exps, rsum = state[i]
                # transpose expS tiles -> expst (bf16 via PSUM)
                for half in range(2):
                    nhalf = NB // 2
                    ttb = ttbp.tile(
                        [128, nhalf * 128], BF16, tag="ttb", name=f"ttb{half}_{h}_{i}"
                    )
                    for jj in range(nhalf):
                        j = half * nhalf + jj
                        nc.tensor.transpose(
                            ttb[:, jj * 128 : (jj + 1) * 128],
                            exps[:, j * 128 : (j + 1) * 128],
                            identb_s[:, :],
                        )
                    nc.vector.tensor_copy(
                        expst[
                            :,
                            half * nhalf : (half + 1) * nhalf,
                            bb * 128 : (bb + 1) * 128,
                        ],
                        ttb[:, :].rearrange("p (j t) -> p j t", j=nhalf),
                    )
                # normalize + write P
                pn = pnp.tile([128, S], F32, tag="pn", name=f"pn_{h}_{i}")
                nc.vector.tensor_scalar(
                    pn[:, :], exps[:, :], rsum[:, :], None, ALU.mult
                )
                nc.sync.dma_start(p_d[h, s0 : s0 + 128, :], pn[:, :])

            def emit_av(g, expst):
                ot = otp_pool.tile([64, 512], F32, tag="ot", name=f"ot_{h}_{g}")
                for j in range(NB):
                    nc.tensor.matmul(
                        ot[:, :],
                        vb[:, j * 64 : (j + 1) * 64],
                        expst[:, j, :],
                        start=(j == 0),
                        stop=(j == NB - 1),
                    )
                otsb = otsbp.tile([64, 512], BF16, tag="otsb", name=f"otsb_{h}_{g}")
                nc.scalar.copy(otsb[:, :], ot[:, :])
                for bb in range(4):
                    i = g * 4 + bb
                    otp = ttbp.tile([128, 64], BF16, tag="ttb", name=f"otp_{h}_{i}")
                    nc.tensor.transpose(
                        otp[:, :],
                        otsb[:, bb * 128 : (bb + 1) * 128],
                        identb_s[0:64, 0:64],
                    )
                    ob = obp.tile([128, 64], F32, tag="ob", name=f"ob_{h}_{i}")
                    rsum_i = state[i][1]
                    nc.vector.tensor_scalar(
                        ob[:, :], otp[:, :], rsum_i[:, :], None, ALU.mult
                    )
                    nc.sync.dma_start(o_d[h, i * 128 : (i + 1) * 128, :], ob[:, :])

            expst_tiles = {}
            for i in range(NB + 1):
                if i < NB:
                    g = i // 4
                    if i % 4 == 0:
                        expst_tiles[g] = expstp.tile(
                            [128, NB, 512], BF16, tag="expst", name=f"expst_{h}_{g}"
                        )
                    emit_scores(i)
                if i >= 1:
                    pi = i - 1
                    pg = pi // 4
                    emit_post(pi, expst_tiles[pg])
                    if pi % 4 == 3:
                        emit_av(pg, expst_tiles[pg])
                        del expst_tiles[pg]
            state.clear()

    nc.compile()
    return nc


_NC_CACHE: dict = {}


def _get_nc(S: int, H: int, D: int):
    key = (S, H, D)
    if key not in _NC_CACHE:
        _NC_CACHE[key] = build_nc(S, H, D)
    return _NC_CACHE[key]


def _run(q, k, v, padding_mask, trace=False, **kwargs):
    """Shard across 8 cores, run, and reassemble full outputs.

    Returns ((result, attention_weights), BassKernelResults).
    """
    q = np.asarray(q)
    k = np.asarray(k)
    v = np.asarray(v)
    padding_mask = np.asarray(padding_mask)
    B, HH, S, D = q.shape
    n_cores = 8
    hper = (B * HH) // n_cores

    nc = _get_nc(S, hper, D)

    qf = q.reshape(B * HH, S, D)
    kf = k.reshape(B * HH, S, D)
    vf = v.reshape(B * HH, S, D)

    in_maps = []
    for c in range(n_cores):
        b = (c * hper) // HH
        in_maps.append(
            {
                "q": np.ascontiguousarray(qf[c * hper : (c + 1) * hper]),
                "k": np.ascontiguousarray(kf[c * hper : (c + 1) * hper]),
                "v": np.ascontiguousarray(vf[c * hper : (c + 1) * hper]),
                "pad": np.ascontiguousarray(
                    padding_mask[b, 0, 0:1, :].astype(np.int32)
                ),
            }
        )

    res = run_bass_kernel_spmd(
        nc, in_maps, core_ids=list(range(n_cores)), trace=trace, **kwargs
    )
    o_full = np.concatenate([r["o"] for r in res.results], axis=0).reshape(
        B, HH, S, D
    )
    p_full = np.concatenate([r["p"] for r in res.results], axis=0).reshape(
        B, HH, S, S
    )
    return (o_full, p_full), res


def kernel(q, k, v, padding_mask):
    (o_full, p_full), _ = _run(q, k, v, padding_mask)
    return (o_full, p_full)


# revision 24
# speedup vs baseline: 2.2320x; 1.0587x over previous
"""Trainium2 Bass kernel for masked multi-head attention (nn_Attention_82695300317344).

Reference computation (per (b, h)):
    scores = (Q @ K^T) / sqrt(64)                       # [S, S]
    combined = triu(ones, 1) + padding_mask             # [S, S] (pad broadcast over rows)
    scores = where(combined == 1, -1e9, scores)         # note: overlap (==2) NOT masked
    P = softmax(scores, axis=-1)                        # attention_weights output
    O = P @ V                                           # result output

Sharding: pure data parallel over the 32 (b, h) pairs; core c owns pairs
[4c, 4c+4). All 4 pairs of a core share one batch's padding row, so the
mask prep is done once per core. No collectives.

Per-core device algorithm (S=2048, 4 heads, all PE work in bf16):
  - Masking is fused into the QK^T matmul via augmented contraction rows:
      KA = [K^T ; -240*pad],  KB = [K^T ; -240*(1-pad)],  QA = [Q^T ; ones].
    Each 512-wide t-chunk picks KA (below/at the diagonal chunk for the
    lower two s-blocks) or KB, so every score matmul is a full N=512 with
    one stationary per s-block; only a <=256-wide strip around the
    diagonal tile gets a precomputed additive correction
    (+-240*(1-2*pad) masked by the 128x128 triangle) added in PSUM by
    VectorE.  exp(scale=1/8) turns -240 into the -30 mask (exp ~ 1e-13,
    vanishing vs the softmax sum, matching where(...,-1e9) to ~1e-10).
  - ScalarE computes expS = exp(scores/8) in bf16 with accum_out giving
    the row sums for free; VectorE reciprocal + 2x-mode tensor_scalar
    produce normalized P tiles, written to DRAM as contiguous 1 MiB DMAs
    (the 64 MiB/core f32 P output is the memory-roofline term).
  - The A@V matmul needs t on partitions: PE transposes 128x128 bf16
    tiles of expS (bf16 stays bf16 in PSUM so the eviction copy runs in
    the DVE 2x packed mode), then V-stationary matmuls produce O^T per
    512-wide s-chunk, transposed back by PE and normalized on eviction.
  - The per-head K^T/Q^T prep (PE transposes of bf16 casts) for head h+1
    is emitted inside head h's block loop; the s-block pipeline is
    software-skewed one block (scores of block i+1 are emitted before the
    transposes of block i) so the PE queue never blocks on ScalarE's exp.

Measured on trn2 (8 cores, neuron-profile): ~310-317 us exec time,
rel err (norm) ~3.8e-3 on result, ~3.0e-3 on attention_weights.
"""

import numpy as np

import concourse.bass as bass
import concourse.bacc as bacc
import concourse.tile as tile
import concourse.mybir as mybir
from concourse.bass_utils import run_bass_kernel_spmd

dt = mybir.dt
F32 = dt.float32
F32R = dt.float32r
BF16 = dt.bfloat16
I32 = dt.int32
AF = mybir.ActivationFunctionType
ALU = mybir.AluOpType

NEG = -240.0  # additive mask before the exp scale of 1/8 -> -30


def build_nc(S: int = 2048, H: int = 4, D: int = 64):
    """Build + compile the per-core Bass graph.

    Per-core I/O:
      q, k, v : [H, S, D] f32   (H flattened (b,h) pairs of this core)
      pad     : [1, S] i32      (this core's batch padding row)
      p (out) : [H, S, S] f32   (attention weights)
      o (out) : [H, S, D] f32   (attention result)
    """
    assert S % 512 == 0 and D == 64
    NB = S // 128   # number of 128-row s-blocks
    NG = NB // 4    # groups of 4 s-blocks (one 512-wide s-chunk each)
    NT = S // 512   # 512-wide t-chunks
    NH = 2 if NT >= 2 else 1    # PSUM score halves per s-block row
    CPH = NT // NH              # 512-chunks per half
    HWID = S // NH              # columns per half

    from contextlib import ExitStack

    nc = bacc.Bacc(
        "TRN2",
        target_bir_lowering=False,
        debug=False,
        enable_asserts=False,
        num_devices=8,
    )

    q_d = nc.dram_tensor("q", [H, S, D], F32, kind="ExternalInput").ap()
    k_d = nc.dram_tensor("k", [H, S, D], F32, kind="ExternalInput").ap()
    v_d = nc.dram_tensor("v", [H, S, D], F32, kind="ExternalInput").ap()
    pad_d = nc.dram_tensor("pad", [1, S], I32, kind="ExternalInput").ap()
    p_d = nc.dram_tensor("p", [H, S, S], F32, kind="ExternalOutput").ap()
    o_d = nc.dram_tensor("o", [H, S, D], F32, kind="ExternalOutput").ap()

    ident_d = nc.inline_tensor(np.eye(128, dtype=np.float32), "identconst").ap()
    tri_d = nc.inline_tensor(
        np.triu(np.ones((128, 128), dtype=np.float32), 1), "triconst"
    ).ap()

    with tile.TileContext(nc) as tc, ExitStack() as ctx:
        # ---------------- pools ----------------
        cons = ctx.enter_context(tc.tile_pool(name="cons", bufs=1))
        natp = ctx.enter_context(tc.tile_pool(name="nat", bufs=2))
        kap = ctx.enter_context(tc.tile_pool(name="kap", bufs=2))
        kbp = ctx.enter_context(tc.tile_pool(name="kbp", bufs=2))
        qap = ctx.enter_context(tc.tile_pool(name="qap", bufs=2))
        vbp = ctx.enter_context(tc.tile_pool(name="vbp", bufs=2))
        expsp = ctx.enter_context(tc.tile_pool(name="expsp", bufs=3))
        pnp = ctx.enter_context(tc.tile_pool(name="pnp", bufs=3))
        expstp = ctx.enter_context(tc.tile_pool(name="expstp", bufs=2))
        sumsp = ctx.enter_context(tc.tile_pool(name="sumsp", bufs=10))
        rsump = ctx.enter_context(tc.tile_pool(name="rsump", bufs=10))
        otsbp = ctx.enter_context(tc.tile_pool(name="otsbp", bufs=2))
        obp = ctx.enter_context(tc.tile_pool(name="obp", bufs=3))
        # PSUM pools: sc 2 slots x 2 banks + ttb 2 x 1 + ot 2 x 1 = 8 banks
        scp = ctx.enter_context(tc.tile_pool(name="scp", bufs=2, space="PSUM"))
        ttbp = ctx.enter_context(tc.tile_pool(name="ttbp", bufs=3, space="PSUM"))
        otp_pool = ctx.enter_context(tc.tile_pool(name="otp", bufs=1, space="PSUM"))

        # ---------------- constants / padding prep ----------------
        ident_s = cons.tile([128, 128], F32, tag="ident")
        nc.sync.dma_start(ident_s[:, :], ident_d[:, :])
        identb_s = cons.tile([128, 128], BF16, tag="identb")
        nc.vector.tensor_copy(identb_s[:, :], ident_s[:, :])
        tri_s = cons.tile([128, 128], F32, tag="tri")
        nc.sync.dma_start(tri_s[:, :], tri_d[:, :])

        padi = cons.tile([1, S], I32, tag="padi")
        nc.sync.dma_start(padi[:, :], pad_d[:, :])
        padf = cons.tile([1, S], F32, tag="padf")
        nc.vector.tensor_copy(padf[:, :], padi[:, :])

        rowA = cons.tile([1, S], BF16, tag="rowA")
        nc.vector.tensor_scalar(rowA[:, :], padf[:, :], NEG, None, ALU.mult)
        rowB = cons.tile([1, S], BF16, tag="rowB")
        nc.vector.tensor_scalar(rowB[:, :], padf[:, :], -NEG, NEG, ALU.mult, ALU.add)

        onesb = cons.tile([1, S], BF16, tag="onesb")
        nc.gpsimd.memset(onesb[:, :], 1.0)
        padfb = cons.tile([1, S], BF16, tag="padfb")
        nc.vector.tensor_copy(padfb[:, :], padf[:, :])

        # padb[128, S]: pad row broadcast to 128 partitions (PE outer product)
        padb = pnp.tile([128, S], F32, tag="pn")
        for c in range(NT):
            pb_ps = scp.tile([128, 1024], F32, tag="scp")
            nc.tensor.matmul(
                pb_ps[:, 0:512],
                onesb[0:1, 0:128],
                padfb[0:1, c * 512 : (c + 1) * 512],
                start=True,
                stop=True,
            )
            nc.vector.tensor_copy(padb[:, c * 512 : (c + 1) * 512], pb_ps[:, 0:512])

        # Diagonal-chunk corrections. The diag 512-chunk g is computed with
        # KB (rowB mask) for blocks bb in {0,1} and KA (rowA) for bb in {2,3};
        # the correction added in PSUM is u[t]=240*(1-2*pad[t]) over t<=s
        # (KB case) or -u[t] over t>s (KA case), restricted to the column
        # range where the applied row is wrong.
        ub = pnp.tile([128, S], F32, tag="pn", name="ubtile")
        nc.vector.tensor_scalar(ub[:, :], padb[:, :], -480.0, 240.0, ALU.mult, ALU.add)
        nub = pnp.tile([128, S], F32, tag="pn", name="nubtile")
        nc.vector.tensor_scalar(nub[:, :], padb[:, :], 480.0, -240.0, ALU.mult, ALU.add)
        ltri = cons.tile([128, 128], F32, tag="ltri")
        nc.vector.tensor_scalar(ltri[:, :], tri_s[:, :], -1.0, 1.0, ALU.mult, ALU.add)
        # fixv layout per group g (offset g*768):
        #   bb=0: [0,128)   = ltri * u[512g:+128]          (targets chunk cols [0,128))
        #   bb=1: [128,384) = [u[512g:+128] | ltri*u[512g+128:+128]]   (cols [0,256))
        #   bb=2: [384,640) = [tri*(-u[512g+256:+128]) | -u[512g+384:+128]] (cols [256,512))
        #   bb=3: [640,768) = tri*(-u[512g+384:+128])      (cols [384,512))
        fixv = cons.tile([128, NG * 768], BF16, tag="fixv")
        for g in range(NG):
            t0 = g * 512
            fo = g * 768
            nc.vector.tensor_mul(
                fixv[:, fo : fo + 128], ltri[:, :], ub[:, t0 : t0 + 128]
            )
            nc.vector.tensor_copy(
                fixv[:, fo + 128 : fo + 256], ub[:, t0 : t0 + 128]
            )
            nc.vector.tensor_mul(
                fixv[:, fo + 256 : fo + 384], ltri[:, :], ub[:, t0 + 128 : t0 + 256]
            )
            nc.vector.tensor_mul(
                fixv[:, fo + 384 : fo + 512], tri_s[:, :], nub[:, t0 + 256 : t0 + 384]
            )
            nc.vector.tensor_copy(
                fixv[:, fo + 512 : fo + 640], nub[:, t0 + 384 : t0 + 512]
            )
            nc.vector.tensor_mul(
                fixv[:, fo + 640 : fo + 768], tri_s[:, :], nub[:, t0 + 384 : t0 + 512]
            )

        # ---------------- per-head main loop ----------------
        for h in range(H):
            # --- K^T prep: KA = [K^T ; rowA], KB = [K^T ; rowB] ---
            knat = natp.tile([128, NB * 64], F32, tag="nat")
            nc.sync.dma_start(
                knat[:, :].rearrange("p (j d) -> p j d", j=NB),
                k_d[h].rearrange("(j p) d -> p j d", p=128),
            )
            knb = natp.tile([128, NB * 64], BF16, tag="natb")
            nc.vector.tensor_copy(knb[:, :], knat[:, :])
            ka = kap.tile([65, S], BF16)
            kb = kbp.tile([65, S], BF16)
            for half in range(2):
                nhalf = NB // 2
                ktp = ttbp.tile([64, nhalf * 128], BF16, tag="ttb")
                for jj in range(nhalf):
                    j = half * nhalf + jj
                    nc.tensor.transpose(
                        ktp[:, jj * 128 : (jj + 1) * 128],
                        knb[:, j * 64 : (j + 1) * 64],
                        identb_s[:, :],
                    )
                dsl = slice(half * nhalf * 128, (half + 1) * nhalf * 128)
                nc.scalar.copy(ka[0:64, dsl], ktp[:, :])
            nc.sync.dma_start(kb[0:64, :], ka[0:64, :])
            nc.sync.dma_start(ka[64:65, :], rowA[:, :])
            nc.sync.dma_start(kb[64:65, :], rowB[:, :])

            # --- Q^T prep: QA = [Q^T ; ones] ---
            qnat = natp.tile([128, NB * 64], F32, tag="nat")
            nc.sync.dma_start(
                qnat[:, :].rearrange("p (j d) -> p j d", j=NB),
                q_d[h].rearrange("(j p) d -> p j d", p=128),
            )
            qnb = natp.tile([128, NB * 64], BF16, tag="natb")
            nc.vector.tensor_copy(qnb[:, :], qnat[:, :])
            qa = qap.tile([65, S], BF16)
            for half in range(2):
                nhalf = NB // 2
                qtp = ttbp.tile([64, nhalf * 128], BF16, tag="ttb")
                for jj in range(nhalf):
                    j = half * nhalf + jj
                    nc.tensor.transpose(
                        qtp[:, jj * 128 : (jj + 1) * 128],
                        qnb[:, j * 64 : (j + 1) * 64],
                        identb_s[:, :],
                    )
                dsl = slice(half * nhalf * 128, (half + 1) * nhalf * 128)
                nc.scalar.copy(qa[0:64, dsl], qtp[:, :])
            nc.sync.dma_start(qa[64:65, :], onesb[0:1, :])

            # --- V load + bf16 cast ---
            vnat = natp.tile([128, NB * 64], F32, tag="nat")
            nc.sync.dma_start(
                vnat[:, :].rearrange("p (j d) -> p j d", j=NB),
                v_d[h].rearrange("(j p) d -> p j d", p=128),
            )
            vb = vbp.tile([128, NB * 64], BF16)
            nc.vector.tensor_copy(vb[:, :], vnat[:, :])

            # --- s-block main loop (1-block software skew on PE) ---
            # per-block state carried across the skew
            state = {}

            def emit_scores(i):
                g, bb = i // 4, i % 4
                s0 = i * 128
                sc = [
                    scp.tile([128, HWID], F32, tag="scp", name=f"sc{hh}_{h}_{i}")
                    for hh in range(NH)
                ]
                qa_full = qa[0:65, s0 : s0 + 128]
                for c in range(NT):
                    half, off = c // CPH, (c % CPH) * 512
                    outc = sc[half][:, off : off + 512]
                    t0 = c * 512
                    use_ka = (c < g) or (c == g and bb >= 2)
                    src_k = ka if use_ka else kb
                    nc.tensor.matmul(
                        outc,
                        qa_full,
                        src_k[0:65, t0 : t0 + 512],
                        start=True,
                        stop=True,
                    )
                # diagonal-chunk correction in PSUM
                FIXO = {0: (0, 128, 0), 1: (128, 256, 0), 2: (384, 256, 256), 3: (640, 128, 384)}
                fo, w, cstart = FIXO[bb]
                fo += g * 768
                dh = g // CPH
                doff = (g % CPH) * 512 + cstart
                for wo in range(0, w, 128):
                    nc.tensor.matmul(
                        sc[dh][:, doff + wo : doff + wo + 128],
                        identb_s[:, :],
                        fixv[:, fo + wo : fo + wo + 128],
                        start=False,
                        stop=True,
                        skip_group_check=True,
                    )
                # exp with accumulated row sums
                exps = expsp.tile([128, S], BF16, tag="exps", name=f"exps_{h}_{i}")
                hsums = [None] * NH
                for hh in sorted(range(NH), key=lambda x: x == dh):
                    hsum = sumsp.tile(
                        [128, 1], F32, tag="hsum", name=f"hsum{hh}_{h}_{i}"
                    )
                    nc.scalar.activation(
                        exps[:, hh * HWID : (hh + 1) * HWID],
                        sc[hh][:, :],
                        AF.Exp,
                        scale=0.125,
                        accum_out=hsum[:, :],
                    )
                    hsums[hh] = hsum
                if NH == 2:
                    sumt = sumsp.tile([128, 1], F32, tag="sumt", name=f"sumt_{h}_{i}")
                    nc.vector.tensor_add(sumt[:, :], hsums[0][:, :], hsums[1][:, :])
                else:
                    sumt = hsums[0]
                rsum = rsump.tile([128, 1], F32, tag="rsum", name=f"rsum_{h}_{i}")
                nc.vector.reciprocal(rsum[:, :], sumt[:, :])
                state[i] = (exps, rsum)

            def emit_post(i, expst):
                g, bb = i // 4, i % 4
                s0 = i * 128
                exps, rsum = state[i]
                # transpose expS tiles -> expst (bf16 via PSUM)
                for half in range(2):
                    nhalf = NB // 2
                    ttb = ttbp.tile(
                        [128, nhalf * 128], BF16, tag="ttb", name=f"ttb{half}_{h}_{i}"
                    )
                    for jj in range(nhalf):
                        j = half * nhalf + jj
                        nc.tensor.transpose(
                            ttb[:, jj * 128 : (jj + 1) * 128],
                            exps[:, j * 128 : (j + 1) * 128],
                            identb_s[:, :],
                        )
                    nc.vector.tensor_copy(
                        expst[
                            :,
                            half * nhalf : (half + 1) * nhalf,
                            bb * 128 : (bb + 1) * 128,
                        ],
                        ttb[:, :].rearrange("p (j t) -> p j t", j=nhalf),
                    )
                # normalize + write P
                pn = pnp.tile([128, S], F32, tag="pn", name=f"pn_{h}_{i}")
                nc.vector.tensor_scalar(
                    pn[:, :], exps[:, :], rsum[:, :], None, ALU.mult
                )
                nc.sync.dma_start(p_d[h, s0 : s0 + 128, :], pn[:, :])

            def emit_av(g, expst):
                ot = otp_pool.tile([64, 512], F32, tag="ot", name=f"ot_{h}_{g}")
                for j in range(NB):
                    nc.tensor.matmul(
                        ot[:, :],
                        vb[:, j * 64 : (j + 1) * 64],
                        expst[:, j, :],
                        start=(j == 0),
                        stop=(j == NB - 1),
                    )
                otsb = otsbp.tile([64, 512], BF16, tag="otsb", name=f"otsb_{h}_{g}")
                nc.scalar.copy(otsb[:, :], ot[:, :])
                for bb in range(4):
                    i = g * 4 + bb
                    otp = ttbp.tile([128, 64], BF16, tag="ttb", name=f"otp_{h}_{i}")
                    nc.tensor.transpose(
                        otp[:, :],
                        otsb[:, bb * 128 : (bb + 1) * 128],
                        identb_s[0:64, 0:64],
                    )
                    ob = obp.tile([128, 64], F32, tag="ob", name=f"ob_{h}_{i}")
                    rsum_i = state[i][1]
                    nc.vector.tensor_scalar(
                        ob[:, :], otp[:, :], rsum_i[:, :], None, ALU.mult
                    )
                    nc.sync.dma_start(o_d[h, i * 128 : (i + 1) * 128, :], ob[:, :])

            expst_tiles = {}
            for i in range(NB + 1):
                if i < NB:
                    g = i // 4
                    if i % 4 == 0:
                        expst_tiles[g] = expstp.tile(
                            [128, NB, 512], BF16, tag="expst", name=f"expst_{h}_{g}"
                        )
                    emit_scores(i)
                if i >= 1:
                    pi = i - 1
                    pg = pi // 4
                    emit_post(pi, expst_tiles[pg])
                if i >= 5 and (i - 5) % 4 == 0:
                    ag = (i - 5) // 4
                    emit_av(ag, expst_tiles[ag])
                    del expst_tiles[ag]
            emit_av(NG - 1, expst_tiles[NG - 1])
            del expst_tiles[NG - 1]
            state.clear()

    nc.compile()
    return nc


_NC_CACHE: dict = {}


def _get_nc(S: int, H: int, D: int):
    key = (S, H, D)
    if key not in _NC_CACHE:
        _NC_CACHE[key] = build_nc(S, H, D)
    return _NC_CACHE[key]


def _run(q, k, v, padding_mask, trace=False, **kwargs):
    """Shard across 8 cores, run, and reassemble full outputs.

    Returns ((result, attention_weights), BassKernelResults).
    """
    q = np.asarray(q)
    k = np.asarray(k)
    v = np.asarray(v)
    padding_mask = np.asarray(padding_mask)
    B, HH, S, D = q.shape
    n_cores = 8
    hper = (B * HH) // n_cores

    nc = _get_nc(S, hper, D)

    qf = q.reshape(B * HH, S, D)
    kf = k.reshape(B * HH, S, D)
    vf = v.reshape(B * HH, S, D)

    in_maps = []
    for c in range(n_cores):
        b = (c * hper) // HH
        in_maps.append(
            {
                "q": np.ascontiguousarray(qf[c * hper : (c + 1) * hper]),
                "k": np.ascontiguousarray(kf[c * hper : (c + 1) * hper]),
                "v": np.ascontiguousarray(vf[c * hper : (c + 1) * hper]),
                "pad": np.ascontiguousarray(
                    padding_mask[b, 0, 0:1, :].astype(np.int32)
                ),
            }
        )

    res = run_bass_kernel_spmd(
        nc, in_maps, core_ids=list(range(n_cores)), trace=trace, **kwargs
    )
    o_full = np.concatenate([r["o"] for r in res.results], axis=0).reshape(
        B, HH, S, D
    )
    p_full = np.concatenate([r["p"] for r in res.results], axis=0).reshape(
        B, HH, S, S
    )
    return (o_full, p_full), res


def kernel(q, k, v, padding_mask):
    (o_full, p_full), _ = _run(q, k, v, padding_mask)
    return (o_full, p_full)
